# revision 1
# baseline (speedup 1.0000x reference)
"""MoE layer (top-2 of 8 experts) Trainium2 kernel, expert-parallel on 8 cores.

Strategy
--------
Host: computes the router (logits -> softmax -> top-2) in float64, builds the
per-expert token dispatch (capacity C with zero-weight padding), gathers and
lays out per-core inputs for DMA-friendly access, and scatter-adds the
per-expert partial outputs back into the full output (the "all-to-all
dispatch/combine" of the sharding hint, done host-side since the contract is
full input -> full output).

Device (per core, expert e): y = (gelu(x @ w1 + b1) @ w2 + b2) * w_combine
for the C tokens routed to the core's expert. Two fp32r GEMMs on the PE
(self-loading matmuls, fp22 multiply / fp32 accumulate), gelu + bias fused on
ScalarE, b2 added via a K=1 ones-vector matmul into the same PSUM accumulation
group, combine-weight applied by ScalarE on PSUM->SBUF evacuation.

Tiling: tokens processed in chunks of 768; per chunk GEMM1 produces
hid [4096, 768] (F on partitions) which stays SBUF-resident, then GEMM2
contracts over F with 6 concurrent PSUM accumulation groups (one per
128-token group) so each streamed w2 tile is reused 6x24 times.
Weights are host-retiled so every DMA lands with >=8KB contiguous
per-partition runs.
"""

import numpy as np

# ---------------------------------------------------------------- constants
B, S, H, F, E, TOP_K = 4, 2048, 1024, 4096, 8, 2
T = B * S
C = 2304          # per-expert token capacity (mean load is T*K/E = 2048)
CHUNK = 768       # tokens per chunk
NCHUNK = C // CHUNK
NCG = 384         # GEMM1 moving-operand width (2 per chunk)
NH = H // 128     # 8 h-blocks
NF = F // 128     # 32 f-tiles
FBLK = 4          # f-tiles per w1 block (512 f-cols)
NFBLK = NF // FBLK
G6 = CHUNK // 128  # 6 concurrent psum groups in GEMM2
NQ = 4            # f-tiles per w2 quad tile
NQUAD = NF // NQ  # 8

_CACHE = {}


def _build_nc(loop_r=None):
    import concourse.mybir as mybir
    from concourse import bacc
    from concourse.tile import TileContext
    from contextlib import ExitStack

    F32 = mybir.dt.float32
    F32R = mybir.dt.float32r
    AFT = mybir.ActivationFunctionType

    nc = bacc.Bacc(None, target_bir_lowering=False)

    # DRAM tensors (host-prepared layouts; see kernel() below)
    xr_d = nc.dram_tensor("xr", [NCHUNK, 128, NH * CHUNK], F32, kind="ExternalInput")
    w1r_d = nc.dram_tensor("w1r", [NFBLK, 128, NH * FBLK * 128], F32, kind="ExternalInput")
    w2r_d = nc.dram_tensor("w2r", [2, NQUAD, 128, NQ * 512], F32, kind="ExternalInput")
    b1_d = nc.dram_tensor("b1", [F], F32, kind="ExternalInput")
    b2_d = nc.dram_tensor("b2", [H], F32, kind="ExternalInput")
    wc_d = nc.dram_tensor("wc", [128, C // 128], F32, kind="ExternalInput")
    ones_d = nc.dram_tensor("ones", [1, 128], F32, kind="ExternalInput")
    y_d = nc.dram_tensor("y", [C, H], F32, kind="ExternalOutput")

    with TileContext(nc) as tc:
        with ExitStack() as stk:
            cpool = stk.enter_context(tc.tile_pool(name="consts", bufs=1))
            xp = stk.enter_context(tc.tile_pool(name="xp", bufs=1))
            w1p = stk.enter_context(tc.tile_pool(name="w1p", bufs=2))
            w2p = stk.enter_context(tc.tile_pool(name="w2p", bufs=3))
            hidp = stk.enter_context(tc.tile_pool(name="hidp", bufs=NF))
            outp = stk.enter_context(tc.tile_pool(name="outp", bufs=4))
            ps1p = stk.enter_context(tc.tile_pool(name="ps1", bufs=2, space="PSUM"))
            ps2p = stk.enter_context(tc.tile_pool(name="ps2", bufs=G6, space="PSUM"))

            b1t = cpool.tile([128, NF], F32, name="b1t")
            b2t = cpool.tile([1, H], F32R, name="b2t")
            wct = cpool.tile([128, C // 128], F32, name="wct")
            ones = cpool.tile([1, 128], F32R, name="ones")
            nc.sync.dma_start(out=b1t, in_=b1_d.rearrange("(t p) -> p t", p=128))
            nc.sync.dma_start(out=b2t, in_=b2_d.rearrange("(o n) -> o n", o=1).bitcast(F32R))
            nc.sync.dma_start(out=wct, in_=wc_d[:, :])
            nc.sync.dma_start(out=ones, in_=ones_d[:, :].bitcast(F32R))

            def body(it):
                for ck in range(NCHUNK):
                    # ---- load x chunk: [128, NH*CHUNK], 24KB/partition runs
                    xb = xp.tile([128, NH * CHUNK], F32R, tag="xb", name=f"xb_{it}_{ck}")
                    nc.sync.dma_start(out=xb, in_=xr_d[ck].bitcast(F32R))

                    # ---- GEMM1: hid[f, c] = gelu(w1.T x + b1), F on partitions
                    hids = []
                    for fb in range(NFBLK):
                        w1b = w1p.tile(
                            [128, NH * FBLK * 128], F32R, tag="w1b",
                            name=f"w1b_{it}_{ck}_{fb}",
                        )
                        nc.sync.dma_start(out=w1b, in_=w1r_d[fb].bitcast(F32R))
                        for fl in range(FBLK):
                            f128 = fb * FBLK + fl
                            hid_t = hidp.tile(
                                [128, CHUNK], F32R, tag="hid",
                                name=f"hid_{it}_{ck}_{f128}",
                            )
                            for ncg in range(CHUNK // NCG):
                                ps = ps1p.tile([128, NCG], F32, tag="ps1", name=f"ps1_{it}_{ck}_{f128}_{ncg}")
                                for h in range(NH):
                                    nc.tensor.matmul(
                                        ps,
                                        lhsT=w1b[:, h * FBLK * 128 + fl * 128 : h * FBLK * 128 + (fl + 1) * 128],
                                        rhs=xb[:, h * CHUNK + ncg * NCG : h * CHUNK + ncg * NCG + NCG],
                                        start=(h == 0),
                                        stop=(h == NH - 1),
                                    )
                                nc.scalar.activation(
                                    hid_t[:, ncg * NCG : (ncg + 1) * NCG],
                                    ps,
                                    AFT.Gelu,
                                    bias=b1t[:, f128 : f128 + 1],
                                )
                            hids.append(hid_t)

                    # ---- GEMM2: y[c, h] = (hid.T w2 + b2) * wc
                    for h5 in range(2):
                        pss = [
                            ps2p.tile([128, 512], F32, tag="ps2", name=f"ps2_{it}_{ck}_{h5}_{g}")
                            for g in range(G6)
                        ]
                        for q in range(NQUAD):
                            w2q = w2p.tile(
                                [128, NQ * 512], F32R, tag="w2q",
                                name=f"w2q_{it}_{ck}_{h5}_{q}",
                            )
                            nc.sync.dma_start(out=w2q, in_=w2r_d[h5, q].bitcast(F32R))
                            for fl in range(NQ):
                                f128 = q * NQ + fl
                                for g in range(G6):
                                    nc.tensor.matmul(
                                        pss[g],
                                        lhsT=hids[f128][:, g * 128 : (g + 1) * 128],
                                        rhs=w2q[:, fl * 512 : (fl + 1) * 512],
                                        start=(f128 == 0),
                                        stop=False,
                                    )
                        for g in range(G6):
                            nc.tensor.matmul(
                                pss[g],
                                lhsT=ones,
                                rhs=b2t[:1, h5 * 512 : (h5 + 1) * 512],
                                start=False,
                                stop=True,
                            )
                            ot = outp.tile([128, 512], F32, tag="ot", name=f"ot_{it}_{ck}_{h5}_{g}")
                            nc.scalar.mul(ot, pss[g], wct[:, ck * G6 + g : ck * G6 + g + 1])
                            nc.sync.dma_start(
                                out=y_d[
                                    ck * CHUNK + g * 128 : ck * CHUNK + (g + 1) * 128,
                                    h5 * 512 : (h5 + 1) * 512,
                                ],
                                in_=ot,
                            )

            if loop_r is None:
                body(0)
            else:
                with tc.For_i(0, loop_r, 1) as _i:
                    body(0)
    nc.compile()
    return nc


def _get_nc(loop_r=None):
    key = ("nc", loop_r)
    if key not in _CACHE:
        _CACHE[key] = _build_nc(loop_r)
    return _CACHE[key]


# ---------------------------------------------------------------- host side
def _route(x2d, router_w):
    """Float64 mirror of the reference router. Returns per-expert padded
    index lists [E, C] and combine weights [E, C]."""
    logits = x2d.astype(np.float64) @ router_w.astype(np.float64).T  # [T, E]
    m = logits.max(axis=1, keepdims=True)
    p = np.exp(logits - m)
    p /= p.sum(axis=1, keepdims=True)
    # top-2 (ties -> lower index, matching jax.lax.top_k)
    order = np.argsort(-p, axis=1, kind="stable")
    top2 = order[:, :TOP_K]  # [T, 2]
    idx = np.zeros((E, C), np.int64)
    wts = np.zeros((E, C), np.float32)
    counts = np.zeros(E, np.int64)
    sel = np.zeros((T, E), bool)
    np.put_along_axis(sel, top2, True, axis=1)
    for e in range(E):
        tok = np.nonzero(sel[:, e])[0]
        n = len(tok)
        if n > C:
            raise RuntimeError(f"expert {e} overflow: {n} > capacity {C}")
        idx[e, :n] = tok
        wts[e, :n] = p[tok, e].astype(np.float32)
        counts[e] = n
    return idx, wts, counts


def _prep_core_inputs(x2d, idx_e, wts_e, w1_e, b1_e, w2_e, b2_e):
    xg = x2d[idx_e]                      # [C, H]
    # xr[ck, p, hb*CHUNK + c] = xg[ck*CHUNK + c, hb*128 + p]
    xr = (
        xg.reshape(NCHUNK, CHUNK, NH, 128)
        .transpose(0, 3, 2, 1)
        .reshape(NCHUNK, 128, NH * CHUNK)
    )
    # w1r[fb, p, h*FBLK*128 + fl*128 + m] = w1[h*128 + p, fb*512 + fl*128 + m]
    w1r = (
        w1_e.reshape(NH, 128, NFBLK, FBLK * 128)
        .transpose(2, 1, 0, 3)
        .reshape(NFBLK, 128, NH * FBLK * 128)
    )
    # w2r[h5, q, p, fl*512 + n] = w2[(q*NQ + fl)*128 + p, h5*512 + n]
    w2r = (
        w2_e.reshape(NQUAD, NQ, 128, 2, 512)
        .transpose(3, 0, 2, 1, 4)
        .reshape(2, NQUAD, 128, NQ * 512)
    )
    wc = np.ascontiguousarray(wts_e.reshape(C // 128, 128).T)  # [128, C/128]
    return {
        "xr": np.ascontiguousarray(xr),
        "w1r": np.ascontiguousarray(w1r),
        "w2r": np.ascontiguousarray(w2r),
        "b1": np.ascontiguousarray(b1_e),
        "b2": np.ascontiguousarray(b2_e),
        "wc": wc,
        "ones": np.ones((1, 128), np.float32),
    }


def kernel(hidden_states, router_w, w1, b1, w2, b2):
    from concourse.bass_utils import run_bass_kernel_spmd

    x2d = np.ascontiguousarray(
        np.asarray(hidden_states, dtype=np.float32).reshape(T, H)
    )
    router_w = np.asarray(router_w, dtype=np.float32)
    w1 = np.asarray(w1, dtype=np.float32)
    b1 = np.asarray(b1, dtype=np.float32)
    w2 = np.asarray(w2, dtype=np.float32)
    b2 = np.asarray(b2, dtype=np.float32)

    idx, wts, counts = _route(x2d, router_w)

    nc = _get_nc()
    in_maps = [
        _prep_core_inputs(x2d, idx[e], wts[e], w1[e], b1[e], w2[e], b2[e])
        for e in range(E)
    ]
    res = run_bass_kernel_spmd(nc, in_maps, core_ids=list(range(E)))

    out = np.zeros((T, H), np.float32)
    for e in range(E):
        n = int(counts[e])
        y = res.results[e]["y"]
        out[idx[e, :n]] += y[:n]
    return out.reshape(B, S, H)


# revision 2
# speedup vs baseline: 6.8776x; 6.8776x over previous
"""MoE layer (top-2 of 8 experts) Trainium2 kernel, expert-parallel on 8 cores.

Strategy
--------
Host: computes the router (logits -> softmax -> top-2) in float64, builds the
per-expert token dispatch (capacity C with zero-weight padding), gathers and
lays out per-core inputs for DMA-friendly access, and scatter-adds the
per-expert partial outputs back into the full output (the "all-to-all
dispatch/combine" of the sharding hint, done host-side since the contract is
full input -> full output).

Device (per core, expert e): y = (gelu(x @ w1 + b1) @ w2 + b2) * w_combine
for the C tokens routed to the core's expert. Two fp32r GEMMs on the PE
(self-loading matmuls, fp22 multiply / fp32 accumulate), gelu + bias fused on
ScalarE, b2 added via a K=1 ones-vector matmul into the same PSUM accumulation
group, combine-weight applied by ScalarE on PSUM->SBUF evacuation.

Tiling: tokens processed in chunks of 768; per chunk GEMM1 produces
hid [4096, 768] (F on partitions) which stays SBUF-resident, then GEMM2
contracts over F with 6 concurrent PSUM accumulation groups (one per
128-token group) so each streamed w2 tile is reused 6x24 times.
Weights are host-retiled so every DMA lands with >=8KB contiguous
per-partition runs.
"""

import numpy as np

# ---------------------------------------------------------------- constants
B, S, H, F, E, TOP_K = 4, 2048, 1024, 4096, 8, 2
T = B * S
C = 2304          # per-expert token capacity (mean load is T*K/E = 2048)
CHUNK = 768       # tokens per chunk
NCHUNK = C // CHUNK
NCG = 384         # GEMM1 moving-operand width (2 per chunk)
NH = H // 128     # 8 h-blocks
NF = F // 128     # 32 f-tiles
FBLK = 4          # f-tiles per w1 block (512 f-cols)
NFBLK = NF // FBLK
G6 = CHUNK // 128  # 6 concurrent psum groups in GEMM2
NQ = 4            # f-tiles per w2 quad tile
NQUAD = NF // NQ  # 8

_CACHE = {}


def _build_nc(loop_r=None):
    import concourse.mybir as mybir
    from concourse import bacc
    from concourse.tile import TileContext
    from contextlib import ExitStack

    F32 = mybir.dt.float32
    F32R = mybir.dt.float32r
    AFT = mybir.ActivationFunctionType

    nc = bacc.Bacc(None, target_bir_lowering=False)

    # DRAM tensors (host-prepared layouts; see kernel() below)
    xr_d = nc.dram_tensor("xr", [NCHUNK, 128, NH * CHUNK], F32, kind="ExternalInput")
    w1r_d = nc.dram_tensor("w1r", [NFBLK, 128, NH * FBLK * 128], F32, kind="ExternalInput")
    w2r_d = nc.dram_tensor("w2r", [2, NQUAD, 128, NQ * 512], F32, kind="ExternalInput")
    b1_d = nc.dram_tensor("b1", [F], F32, kind="ExternalInput")
    b2_d = nc.dram_tensor("b2", [H], F32, kind="ExternalInput")
    wc_d = nc.dram_tensor("wc", [128, C // 128], F32, kind="ExternalInput")
    y_d = nc.dram_tensor("y", [C, H], F32, kind="ExternalOutput")

    with TileContext(nc) as tc:
        with ExitStack() as stk:
            cpool = stk.enter_context(tc.tile_pool(name="consts", bufs=1))
            xp = stk.enter_context(tc.tile_pool(name="xp", bufs=1))
            w1p = stk.enter_context(tc.tile_pool(name="w1p", bufs=2))
            w2p = stk.enter_context(tc.tile_pool(name="w2p", bufs=3))
            hidp = stk.enter_context(tc.tile_pool(name="hidp", bufs=NF))
            outp = stk.enter_context(tc.tile_pool(name="outp", bufs=4))
            ps1p = stk.enter_context(tc.tile_pool(name="ps1", bufs=2, space="PSUM"))
            ps2p = stk.enter_context(tc.tile_pool(name="ps2", bufs=G6, space="PSUM"))

            b1t = cpool.tile([128, NF], F32, name="b1t")
            b2bc = cpool.tile([128, H], F32, name="b2bc")
            wct = cpool.tile([128, C // 128], F32, name="wct")
            nc.sync.dma_start(out=b1t, in_=b1_d.rearrange("(t p) -> p t", p=128))
            nc.sync.dma_start(
                out=b2bc,
                in_=b2_d.rearrange("(o n) -> o n", o=1).partition_broadcast(128),
            )
            nc.sync.dma_start(out=wct, in_=wc_d[:, :])

            def body(it):
                for ck in range(NCHUNK):
                    # ---- load x chunk: [128, NH*CHUNK], 24KB/partition runs
                    xb = xp.tile([128, NH * CHUNK], F32R, tag="xb", name=f"xb_{it}_{ck}")
                    nc.sync.dma_start(out=xb, in_=xr_d[ck].bitcast(F32R))

                    # ---- GEMM1: hid[f, c] = gelu(w1.T x + b1), F on partitions
                    hids = []
                    for fb in range(NFBLK):
                        w1b = w1p.tile(
                            [128, NH * FBLK * 128], F32R, tag="w1b",
                            name=f"w1b_{it}_{ck}_{fb}",
                        )
                        nc.sync.dma_start(out=w1b, in_=w1r_d[fb].bitcast(F32R))
                        for fl in range(FBLK):
                            f128 = fb * FBLK + fl
                            hid_t = hidp.tile(
                                [128, CHUNK], F32R, tag="hid",
                                name=f"hid_{it}_{ck}_{f128}",
                            )
                            for ncg in range(CHUNK // NCG):
                                ps = ps1p.tile([128, NCG], F32, tag="ps1", name=f"ps1_{it}_{ck}_{f128}_{ncg}")
                                for h in range(NH):
                                    nc.tensor.matmul(
                                        ps,
                                        lhsT=w1b[:, h * FBLK * 128 + fl * 128 : h * FBLK * 128 + (fl + 1) * 128],
                                        rhs=xb[:, h * CHUNK + ncg * NCG : h * CHUNK + ncg * NCG + NCG],
                                        start=(h == 0),
                                        stop=(h == NH - 1),
                                    )
                                nc.scalar.activation(
                                    hid_t[:, ncg * NCG : (ncg + 1) * NCG],
                                    ps,
                                    AFT.Gelu,
                                    bias=b1t[:, f128 : f128 + 1],
                                )
                            hids.append(hid_t)

                    # ---- GEMM2: y[c, h] = (hid.T w2 + b2) * wc
                    for h5 in range(2):
                        pss = [
                            ps2p.tile([128, 512], F32, tag="ps2", name=f"ps2_{it}_{ck}_{h5}_{g}")
                            for g in range(G6)
                        ]
                        for q in range(NQUAD):
                            w2q = w2p.tile(
                                [128, NQ * 512], F32R, tag="w2q",
                                name=f"w2q_{it}_{ck}_{h5}_{q}",
                            )
                            nc.sync.dma_start(out=w2q, in_=w2r_d[h5, q].bitcast(F32R))
                            for fl in range(NQ):
                                f128 = q * NQ + fl
                                for g in range(G6):
                                    nc.tensor.matmul(
                                        pss[g],
                                        lhsT=hids[f128][:, g * 128 : (g + 1) * 128],
                                        rhs=w2q[:, fl * 512 : (fl + 1) * 512],
                                        start=(f128 == 0),
                                        stop=(f128 == NF - 1),
                                    )
                        for g in range(G6):
                            ot = outp.tile([128, 512], F32, tag="ot", name=f"ot_{it}_{ck}_{h5}_{g}")
                            nc.vector.tensor_add(
                                ot, pss[g], b2bc[:, h5 * 512 : (h5 + 1) * 512]
                            )
                            nc.scalar.mul(ot, ot, wct[:, ck * G6 + g : ck * G6 + g + 1])
                            nc.sync.dma_start(
                                out=y_d[
                                    ck * CHUNK + g * 128 : ck * CHUNK + (g + 1) * 128,
                                    h5 * 512 : (h5 + 1) * 512,
                                ],
                                in_=ot,
                            )

            if loop_r is None:
                body(0)
            else:
                with tc.For_i(0, loop_r, 1) as _i:
                    body(0)
    nc.compile()
    return nc


def _get_nc(loop_r=None):
    key = ("nc", loop_r)
    if key not in _CACHE:
        _CACHE[key] = _build_nc(loop_r)
    return _CACHE[key]


# ---------------------------------------------------------------- host side
def _route(x2d, router_w):
    """Float64 mirror of the reference router. Returns per-expert padded
    index lists [E, C] and combine weights [E, C]."""
    logits = x2d.astype(np.float64) @ router_w.astype(np.float64).T  # [T, E]
    m = logits.max(axis=1, keepdims=True)
    p = np.exp(logits - m)
    p /= p.sum(axis=1, keepdims=True)
    # top-2 (ties -> lower index, matching jax.lax.top_k)
    order = np.argsort(-p, axis=1, kind="stable")
    top2 = order[:, :TOP_K]  # [T, 2]
    idx = np.zeros((E, C), np.int64)
    wts = np.zeros((E, C), np.float32)
    counts = np.zeros(E, np.int64)
    sel = np.zeros((T, E), bool)
    np.put_along_axis(sel, top2, True, axis=1)
    for e in range(E):
        tok = np.nonzero(sel[:, e])[0]
        n = len(tok)
        if n > C:
            raise RuntimeError(f"expert {e} overflow: {n} > capacity {C}")
        idx[e, :n] = tok
        wts[e, :n] = p[tok, e].astype(np.float32)
        counts[e] = n
    return idx, wts, counts


def _prep_core_inputs(x2d, idx_e, wts_e, w1_e, b1_e, w2_e, b2_e):
    xg = x2d[idx_e]                      # [C, H]
    # xr[ck, p, hb*CHUNK + c] = xg[ck*CHUNK + c, hb*128 + p]
    xr = (
        xg.reshape(NCHUNK, CHUNK, NH, 128)
        .transpose(0, 3, 2, 1)
        .reshape(NCHUNK, 128, NH * CHUNK)
    )
    # w1r[fb, p, h*FBLK*128 + fl*128 + m] = w1[h*128 + p, fb*512 + fl*128 + m]
    w1r = (
        w1_e.reshape(NH, 128, NFBLK, FBLK * 128)
        .transpose(2, 1, 0, 3)
        .reshape(NFBLK, 128, NH * FBLK * 128)
    )
    # w2r[h5, q, p, fl*512 + n] = w2[(q*NQ + fl)*128 + p, h5*512 + n]
    w2r = (
        w2_e.reshape(NQUAD, NQ, 128, 2, 512)
        .transpose(3, 0, 2, 1, 4)
        .reshape(2, NQUAD, 128, NQ * 512)
    )
    wc = np.ascontiguousarray(wts_e.reshape(C // 128, 128).T)  # [128, C/128]
    return {
        "xr": np.ascontiguousarray(xr),
        "w1r": np.ascontiguousarray(w1r),
        "w2r": np.ascontiguousarray(w2r),
        "b1": np.ascontiguousarray(b1_e),
        "b2": np.ascontiguousarray(b2_e),
        "wc": wc,
    }


def kernel(hidden_states, router_w, w1, b1, w2, b2):
    from concourse.bass_utils import run_bass_kernel_spmd

    x2d = np.ascontiguousarray(
        np.asarray(hidden_states, dtype=np.float32).reshape(T, H)
    )
    router_w = np.asarray(router_w, dtype=np.float32)
    w1 = np.asarray(w1, dtype=np.float32)
    b1 = np.asarray(b1, dtype=np.float32)
    w2 = np.asarray(w2, dtype=np.float32)
    b2 = np.asarray(b2, dtype=np.float32)

    idx, wts, counts = _route(x2d, router_w)

    nc = _get_nc()
    in_maps = [
        _prep_core_inputs(x2d, idx[e], wts[e], w1[e], b1[e], w2[e], b2[e])
        for e in range(E)
    ]
    res = run_bass_kernel_spmd(nc, in_maps, core_ids=list(range(E)))

    out = np.zeros((T, H), np.float32)
    for e in range(E):
        n = int(counts[e])
        y = res.results[e]["y"]
        out[idx[e, :n]] += y[:n]
    return out.reshape(B, S, H)
